# revision 1
# baseline (speedup 1.0000x reference)
"""GAT-style graph encoder on 8 trn2 NeuronCores.

Reference computation (per exercise row i over kc nodes j):
    kc_Wh = kc_h @ W1; ex_Wh = ex_h @ W1
    e[i,j] = leaky_relu(ex_Wh[i]@a1 + kc_Wh[j]@a2, 0.2)
    att = softmax(where(adj>0, e, -9e15), axis=1)
    new_kc = att @ kc_Wh; ex_Eh = ex_h @ E
    out = elu(concat([new_kc, new_kc*ex_Eh]) @ rd_w.T + rd_b)

Strategy: row-shard exercises over 8 cores (1250 rows each, padded to 1280).
On-chip everything lives in a transposed [kc_or_feature, exercise] layout so
softmax numerator/denominator are plain PE matmuls contracting over the kc
partition axis -- no on-chip transposes.  Masking is a multiply (adj is 0/1)
on the exp'd logits; since logits are bounded (|e| <~ 15) the softmax is
computed without max-subtraction, exactly matching reference semantics to
f32 roundoff.  ex_a1 enters via the per-partition broadcast tile, kc_a2 via
the activation bias port, both folded through W1 on the host (weight-only
algebra: ex_Wh@a1 == ex_h@(W1@a1)).
"""

import ml_dtypes
import numpy as np

import concourse.bacc as bacc
import concourse.bass as bass
import concourse.mybir as mybir
from concourse.alu_op_type import AluOpType
from concourse.bass_utils import run_bass_kernel_spmd
from concourse.tile import TileContext

F32 = mybir.dt.float32
F32R = mybir.dt.float32r
BF16 = mybir.dt.bfloat16
AF = mybir.ActivationFunctionType

P = 128
D = 256                    # feature dim
NKC = 2048                 # padded kc count (2000 real)
KCH = NKC // P             # 16 kc chunks
M = 1280                   # padded exercise rows per core (1250 real)
MBS = (512, 512, 256)      # m blocks (>=256 keeps float32r at 1 cyc/row)
MOFF = (0, 512, 1024)
NCORES = 8
ROWS = 1250
N_E = 10000
ALPHA = 0.2
# A: 0/1 multiply-mask (ACT leaky+exp, DVE mask)
# B: fold, Pool tt, ACT leaky | C: fold, DVE tt, ACT leaky
# D: fold, Pool tt, DVE leaky | E: fold, DVE tt, DVE leaky
VARIANTS = ("B", "E", "A", "D", "B", "C", "A", "D")


def _build():
    nc = bacc.Bacc("TRN2", target_bir_lowering=False, debug=False,
                   num_devices=NCORES)
    exT = nc.declare_dram_parameter("exT", [2 * P, M], F32R, isOutput=False)
    adjT = nc.declare_dram_parameter("adjT", [NKC, M], BF16, isOutput=False)
    kcT = nc.declare_dram_parameter("kcT", [2 * P, NKC], F32R, isOutput=False)
    W1e = nc.declare_dram_parameter("W1e", [2 * P, D + 2], F32R, isOutput=False)
    w1a1 = nc.declare_dram_parameter("w1a1", [2 * P, 1], F32R, isOutput=False)
    Em = nc.declare_dram_parameter("Em", [2 * P, D], F32R, isOutput=False)
    rdwT = nc.declare_dram_parameter("rdwT", [4 * P, D], F32R, isOutput=False)
    rdb = nc.declare_dram_parameter("rdb", [2 * P, 1], F32, isOutput=False)
    outT = nc.declare_dram_parameter("outT", [2 * P, M], F32, isOutput=True)

    with TileContext(nc) as tc:
        with tc.tile_pool(name="const", bufs=1) as cpool, \
             tc.tile_pool(name="acc_ps", bufs=1, space="PSUM") as apool, \
             tc.tile_pool(name="out_ps", bufs=1, space="PSUM") as opool, \
             tc.tile_pool(name="mwork", bufs=8) as mpool, \
             tc.tile_pool(name="post", bufs=2) as qpool:
            kcT_sb, W1e_sb, Em_sb, w1a1_sb, rdb_sb, exT_sb = [], [], [], [], [], []
            for c in range(2):
                t = cpool.tile([P, NKC], F32R, tag=f"kcT{c}")
                nc.sync.dma_start(out=t[:], in_=kcT[c * P:(c + 1) * P, :])
                kcT_sb.append(t)
                t = cpool.tile([P, D + 2], F32R, tag=f"W1e{c}")
                nc.sync.dma_start(out=t[:], in_=W1e[c * P:(c + 1) * P, :])
                W1e_sb.append(t)
                t = cpool.tile([P, D], F32R, tag=f"Em{c}")
                nc.sync.dma_start(out=t[:], in_=Em[c * P:(c + 1) * P, :])
                Em_sb.append(t)
                t = cpool.tile([P, 1], F32R, tag=f"w1a1{c}")
                nc.sync.dma_start(out=t[:], in_=w1a1[c * P:(c + 1) * P, :])
                w1a1_sb.append(t)
                t = cpool.tile([P, 1], F32, tag=f"rdb{c}")
                nc.sync.dma_start(out=t[:], in_=rdb[c * P:(c + 1) * P, :])
                rdb_sb.append(t)
                t = cpool.tile([P, M], F32R, tag=f"exT{c}")
                nc.sync.dma_start(out=t[:], in_=exT[c * P:(c + 1) * P, :])
                exT_sb.append(t)
            rdwT_sb = []
            for dd in range(4):
                t = cpool.tile([P, D], F32R, tag=f"rdwT{dd}")
                nc.sync.dma_start(out=t[:], in_=rdwT[dd * P:(dd + 1) * P, :])
                rdwT_sb.append(t)
            ones1f = cpool.tile([1, P], F32, tag="ones1f")
            nc.vector.memset(ones1f[:], 1.0)
            ones1 = cpool.tile([1, P], F32R, tag="ones1")
            nc.scalar.copy(ones1[:], ones1f[:])
            ones128f = cpool.tile([P, 1], F32, tag="ones128f")
            nc.vector.memset(ones128f[:], 1.0)
            ones128 = cpool.tile([P, 1], F32R, tag="ones128")
            nc.scalar.copy(ones128[:], ones128f[:])

            # ---- setup (emitted in dependency-criticality order:
            # exa1b gates every main-loop block, kcWh[kk] gates chunk kk,
            # exEhT is needed only at the post stage of block 0)
            kcWh, kca2 = [], []
            exa1b = cpool.tile([P, M], F32, tag="exa1b")
            exa1_sb = cpool.tile([1, M], F32R, tag="exa1_sb")
            exEhT = [cpool.tile([P, M], F32, tag=f"exEhT{d}", name=f"exEhT{d}")
                     for d in range(2)]
            with tc.tile_pool(name="setup_ps", bufs=2, space="PSUM") as spool:
                for b in range(3):
                    ms = slice(MOFF[b], MOFF[b] + MBS[b])
                    ps = spool.tile([1, MBS[b]], F32, tag="misc_ps",
                                    name=f"row_ps{b}")
                    for c in range(2):
                        nc.tensor.matmul(ps[:], w1a1_sb[c][:],
                                         exT_sb[c][:, ms],
                                         start=(c == 0), stop=(c == 1))
                    nc.vector.tensor_copy(exa1_sb[:, ms], ps[:])
                    psb = spool.tile([P, MBS[b]], F32, tag="misc_ps",
                                     name=f"bc_ps{b}")
                    nc.tensor.matmul(psb[:], ones1[:], exa1_sb[:, ms],
                                     start=True, stop=True)
                    nc.vector.tensor_copy(exa1b[:, ms], psb[:])
                for kk in range(KCH):
                    ps = spool.tile([P, D + 2], F32, tag="kcwh_ps")
                    for c in range(2):
                        nc.tensor.matmul(
                            ps[:], kcT_sb[c][:, kk * P:(kk + 1) * P],
                            W1e_sb[c][:], start=(c == 0), stop=(c == 1))
                    t = cpool.tile([P, D], F32R, tag=f"kcWh{kk}",
                                   name=f"kcWh{kk}")
                    if kk % 2 == 0:
                        nc.scalar.copy(t[:], ps[:, 0:D])
                    else:
                        nc.vector.tensor_copy(t[:], ps[:, 0:D])
                    kcWh.append(t)
                    tb = cpool.tile([P, 1], F32, tag=f"kca2_{kk}",
                                    name=f"kca2_{kk}")
                    nc.scalar.copy(tb[:], ps[:, D:D + 1])
                    kca2.append(tb)
                for d in range(2):
                    for b in range(3):
                        ms = slice(MOFF[b], MOFF[b] + MBS[b])
                        pse = spool.tile([P, MBS[b]], F32, tag="misc_ps",
                                         name=f"eh_ps{b}_{d}")
                        for c in range(2):
                            nc.tensor.matmul(
                                pse[:], Em_sb[c][:, d * P:(d + 1) * P],
                                exT_sb[c][:, ms], start=(c == 0), stop=(c == 1))
                        nc.scalar.copy(exEhT[d][:, ms], pse[:])

            # ---- main: masked softmax attention + aggregation + readout.
            # adjT row encoding is per-chunk (host-matched): chunks with
            # kk % 4 == 2 carry adj as 0/1 (multiply mask); all others carry
            # 100*(adj-1), i.e. 0 / -100, folded into the logits so that
            # leaky(-100+s) -> exp ~ 2e-9 ~ 0.
            for b in range(3):
                mb = MBS[b]
                ms = slice(MOFF[b], MOFF[b] + mb)
                n0 = apool.tile([P, mb], F32, tag="n0")
                n1 = apool.tile([P, mb], F32, tag="n1")
                sS = apool.tile([1, mb], F32, tag="sS")
                for kk in range(KCH):
                    adjf = mpool.tile([P, mb], BF16, tag="adjf", bufs=12)
                    nc.sync.dma_start(out=adjf[:],
                                      in_=adjT[kk * P:(kk + 1) * P, ms])
                    # 8-chunk rotation balancing ACT/DVE/Pool; see VARIANTS
                    v = VARIANTS[kk % 8]
                    ptm = mpool.tile([P, mb], F32R, tag="ptm")
                    if v == "A":  # multiply-mask: leaky+exp ACT, mask DVE
                        et = mpool.tile([P, mb], F32, tag="et")
                        nc.scalar.activation(et[:], exa1b[:, ms], AF.Prelu,
                                             bias=kca2[kk][:], alpha=ALPHA)
                        pt = mpool.tile([P, mb], F32, tag="pt")
                        nc.scalar.activation(pt[:], et[:], AF.Exp)
                        nc.vector.tensor_mul(ptm[:], pt[:], adjf[:])
                    else:         # logit-fold variants
                        tt_eng = nc.gpsimd if v in ("B", "D") else nc.vector
                        tmp = mpool.tile([P, mb], F32, tag="tmp")
                        tt_eng.tensor_add(tmp[:], adjf[:], exa1b[:, ms])
                        et = mpool.tile([P, mb], F32, tag="et")
                        if v in ("B", "C"):   # leaky on ACT
                            nc.scalar.activation(et[:], tmp[:], AF.Prelu,
                                                 bias=kca2[kk][:], alpha=ALPHA)
                        else:                 # leaky on DVE
                            s02 = mpool.tile([P, mb], F32, tag="s02")
                            nc.vector.tensor_scalar(
                                s02[:], tmp[:], kca2[kk][:], ALPHA,
                                AluOpType.add, AluOpType.mult)
                            nc.vector.scalar_tensor_tensor(
                                et[:], tmp[:], kca2[kk][:], s02[:],
                                AluOpType.add, AluOpType.max)
                        nc.scalar.activation(ptm[:], et[:], AF.Exp)
                    st, sp = (kk == 0), (kk == KCH - 1)
                    nc.tensor.matmul(n0[:], kcWh[kk][:, 0:P], ptm[:],
                                     start=st, stop=sp)
                    nc.tensor.matmul(n1[:], kcWh[kk][:, P:2 * P], ptm[:],
                                     start=st, stop=sp)
                    nc.tensor.matmul(sS[:], ones128[:], ptm[:],
                                     start=st, stop=sp)
                srow = qpool.tile([1, mb], F32R, tag="srow")
                with nc.allow_low_precision(reason="f32r storage is full f32"):
                    nc.vector.reciprocal(srow[:], sS[:])
                sbps = opool.tile([P, mb], F32, tag="u")
                nc.tensor.matmul(sbps[:], ones1[:], srow[:],
                                 start=True, stop=True)
                sinvb = qpool.tile([P, mb], F32, tag="sinvb")
                nc.vector.tensor_copy(sinvb[:], sbps[:])
                nk0 = qpool.tile([P, mb], F32R, tag="nk0")
                nc.vector.tensor_mul(nk0[:], n0[:], sinvb[:])
                nk1 = qpool.tile([P, mb], F32R, tag="nk1")
                nc.vector.tensor_mul(nk1[:], n1[:], sinvb[:])
                t0 = qpool.tile([P, mb], F32R, tag="t0")
                nc.gpsimd.tensor_mul(t0[:], nk0[:], exEhT[0][:, ms])
                t1 = qpool.tile([P, mb], F32R, tag="t1")
                nc.gpsimd.tensor_mul(t1[:], nk1[:], exEhT[1][:, ms])
                feat = [nk0, nk1, t0, t1]
                for oo in range(2):
                    ups = opool.tile([P, mb], F32, tag="u")
                    for dd in range(4):
                        nc.tensor.matmul(
                            ups[:], rdwT_sb[dd][:, oo * P:(oo + 1) * P],
                            feat[dd][:], start=(dd == 0), stop=(dd == 3))
                    # elu(x) = max(x,0) + exp(min(x,0)) - 1,  x = ups + rd_b
                    tmin = qpool.tile([P, mb], F32, tag="tmin")
                    nc.vector.tensor_scalar(tmin[:], ups[:], rdb_sb[oo][:],
                                            0.0, AluOpType.add, AluOpType.min)
                    eneg = qpool.tile([P, mb], F32, tag="eneg")
                    nc.scalar.activation(eneg[:], tmin[:], AF.Exp)
                    tmax = qpool.tile([P, mb], F32, tag="tmax")
                    nc.vector.tensor_scalar(tmax[:], ups[:], rdb_sb[oo][:],
                                            0.0, AluOpType.add, AluOpType.max)
                    res = qpool.tile([P, mb], F32, tag="res")
                    nc.vector.scalar_tensor_tensor(res[:], tmax[:], -1.0,
                                                   eneg[:], AluOpType.add,
                                                   AluOpType.add)
                    nc.sync.dma_start(out=outT[oo * P:(oo + 1) * P, ms],
                                      in_=res[:])
    nc.finalize()
    return nc


_PROGRAM = None


def _get_program():
    global _PROGRAM
    if _PROGRAM is None:
        _PROGRAM = _build()
    return _PROGRAM


def _in_maps(exercise_h, kc_h, adj, W1, E, a, rd_w, rd_b):
    f = np.float32
    a1 = np.ascontiguousarray(a[:D, 0], dtype=f)
    a2 = np.ascontiguousarray(a[D:, 0], dtype=f)
    W1 = np.asarray(W1, dtype=f)
    w1a2 = W1 @ a2
    W1e = np.concatenate([W1, w1a2[:, None],
                          np.zeros((D, 1), f)], axis=1)      # [256, 258]
    w1a1 = (W1 @ a1)[:, None]                                 # [256, 1]
    kcT = np.zeros((2 * P, NKC), dtype=f)
    kcT[:, :2000] = np.asarray(kc_h, dtype=f).T
    Em = np.ascontiguousarray(np.asarray(E, dtype=f))
    rdwT = np.ascontiguousarray(np.asarray(rd_w, dtype=f).T)  # [512, 256]
    rdb = np.asarray(rd_b, dtype=f)[:, None]                  # [256, 1]
    shared = {"kcT": kcT, "W1e": np.ascontiguousarray(W1e),
              "w1a1": np.ascontiguousarray(w1a1), "Em": Em,
              "rdwT": rdwT, "rdb": np.ascontiguousarray(rdb)}
    maps = []
    for c in range(NCORES):
        sl = slice(c * ROWS, (c + 1) * ROWS)
        exT_c = np.zeros((2 * P, M), dtype=f)
        exT_c[:, :ROWS] = np.asarray(exercise_h[sl], dtype=f).T
        adjx = np.asarray(adj[sl], dtype=np.float32).T  # [2000, 1250] of 0/1
        adjT_c = np.zeros((NKC, M), dtype=ml_dtypes.bfloat16)
        for kk in range(KCH):
            rs = slice(kk * P, (kk + 1) * P)
            blk = np.zeros((P, M), dtype=np.float32)
            nreal = max(0, min(2000 - kk * P, P))
            if VARIANTS[kk % 8] == "A":   # multiply-mask chunk: 0/1
                blk[:nreal, :ROWS] = adjx[kk * P:kk * P + nreal]
                blk[:nreal, ROWS:] = 1.0   # pad rows finite
                blk[nreal:, :] = 0.0       # pad kc nodes masked out
            else:                 # logit-fold chunk: 0/-100, pad kc = -100
                blk[:nreal, :ROWS] = (adjx[kk * P:kk * P + nreal] - 1.0) * 100.0
                blk[:nreal, ROWS:] = 0.0
                blk[nreal:, :] = -100.0
            adjT_c[rs] = blk
        del adjx
        maps.append({"exT": exT_c, "adjT": adjT_c, **shared})
    return maps


def kernel(exercise_h, kc_h, adj, W1, E, a, rd_w, rd_b):
    nc = _get_program()
    maps = _in_maps(exercise_h, kc_h, adj, W1, E, a, rd_w, rd_b)
    res = run_bass_kernel_spmd(nc, maps, list(range(NCORES))).results
    out = np.empty((N_E, D), dtype=np.float32)
    for c in range(NCORES):
        out[c * ROWS:(c + 1) * ROWS] = res[c]["outT"][:, :ROWS].T
    return out



# revision 32
# speedup vs baseline: 1.3179x; 1.3179x over previous
"""GAT-style graph encoder on 8 trn2 NeuronCores.

Reference computation (per exercise row i over kc nodes j):
    kc_Wh = kc_h @ W1; ex_Wh = ex_h @ W1
    e[i,j] = leaky_relu(ex_Wh[i]@a1 + kc_Wh[j]@a2, 0.2)
    att = softmax(where(adj>0, e, -9e15), axis=1)
    new_kc = att @ kc_Wh; ex_Eh = ex_h @ E
    out = elu(concat([new_kc, new_kc*ex_Eh]) @ rd_w.T + rd_b)

Strategy: row-shard exercises over 8 cores (1250 cols each, no padding).
On-chip layout is transposed ([kc_or_feature, exercise]) so softmax
numerator/denominator are PE matmuls contracting over the kc partition
axis.  Masking is additive (adj shipped as a 0/-100 bf16 logit fold);
softmax runs without max-subtraction (logits bounded, exp in f32->bf16).
All operands are bf16 (rel-err budget ~0.7% << 2e-2 tolerance).

Per kc chunk-pair (2x128 kc rows x 1250 ex cols):
  - one 2500-wide DMA for the fold mask (8 adj loads total; HWDGE is a
    serial 625ns/DMA device, so few fat DMAs beat many thin ones)
  - route A: fold-add on DVE tt (2x bf16 mode) or Pool tt (one half
    each), leaky via ACT Prelu with the kca2 bias port
  - route D: DVE stt carries exa1b+kca2+fold, second DVE stt the leaky
    ((x*0.2) max x); ACT only does exp.  A/D mix balances ACT vs DVE.
  - one 2500-wide ACT Exp into bf16 ptm tiles (last pairs split in two
    to shorten the tail)
  - 3 bf16 matmul streams (n0, n1, denominator) accumulate psum blocks
    {0,1} inline; block {2} is a second pass over resident ptm tiles so
    its PE work overlaps the posts of blocks {0,1}.  The 3 denominator
    rows share one psum bank at partition offsets 0/32/64.
PE p-state is warmed with junk matmuls at t=0 (full 2.4GHz clock needs
~3us of continuous busy).  Post stage normalizes late (1/s applied to
the readout psum), elu(y) = min(exp(y),1) + (relu(y)-1) with engine-
legal op placement (Pool cannot touch PSUM or run TensorScalarPtr).
Setup copies and post ops are spread ACT/DVE/Pool to balance occupancy.
HW exec (TimelineSim): 62.8us vs 81.8us baseline; rel err 6.6e-3.
"""

import ml_dtypes
import numpy as np

import concourse.bacc as bacc
import concourse.mybir as mybir
from concourse.alu_op_type import AluOpType
from concourse.bass_utils import run_bass_kernel_spmd
from concourse.tile import TileContext

F32 = mybir.dt.float32
F32R = mybir.dt.float32r
BF16 = mybir.dt.bfloat16
AF = mybir.ActivationFunctionType

P = 128
D = 256
NKC = 2048
KCH = 16                    # kc chunks of 128
NPAIR = 8                   # chunk pairs
M = 1250                    # exercise cols per core (no padding)
MBS = (512, 512, 226)
MOFF = (0, 512, 1024)
NCORES = 8
ROWS = 1250
N_E = 10000
ALPHA = 0.2
# Per-pair elementwise route: A = ACT Prelu carries kca2+leaky (DVE does
# the fold add); D = DVE stt carries fold+kca2 and a second stt the leaky
# (ACT only does exp).  Mix balances ACT vs DVE occupancy.
ROUTES = ("A", "A", "A", "A", "A", "D", "A", "D")


def _build():
    nc = bacc.Bacc("TRN2", target_bir_lowering=False, debug=False,
                   num_devices=NCORES)
    exT = nc.declare_dram_parameter("exT", [2 * P, M], BF16, isOutput=False)
    adjC = nc.declare_dram_parameter("adjC", [P, KCH * M], BF16,
                                     isOutput=False)
    kcT = nc.declare_dram_parameter("kcT", [2 * P, NKC], BF16, isOutput=False)
    W1e = nc.declare_dram_parameter("W1e", [2 * P, D + 2], BF16,
                                    isOutput=False)
    Em = nc.declare_dram_parameter("Em", [2 * P, D], BF16, isOutput=False)
    rdcat = nc.declare_dram_parameter("rdcat", [P, 4 * D], BF16,
                                      isOutput=False)
    rdbs = nc.declare_dram_parameter("rdbs", [P, 4], F32, isOutput=False)
    outT = nc.declare_dram_parameter("outT", [2 * P, M], BF16, isOutput=True)

    with TileContext(nc) as tc:
        with tc.tile_pool(name="const", bufs=1) as cpool, \
             tc.tile_pool(name="mwork", bufs=3) as mpool, \
             tc.tile_pool(name="post", bufs=2) as qpool:
            # ---- const loads (ordered so compute can start early:
            # exT/W1e gate exa1b; first adj pairs gate the elementwise)
            kcT_sb, W1e_sb, Em_sb, exT_sb = [], [], [], []
            for c in range(2):
                t = cpool.tile([P, M], BF16, tag=f"exT{c}")
                nc.sync.dma_start(out=t[:], in_=exT[c * P:(c + 1) * P, :])
                exT_sb.append(t)
                t = cpool.tile([P, D + 2], BF16, tag=f"W1e{c}")
                nc.sync.dma_start(out=t[:], in_=W1e[c * P:(c + 1) * P, :])
                W1e_sb.append(t)
            H = NKC // 2
            for c in range(2):
                t = cpool.tile([P, NKC], BF16, tag=f"kcT{c}")
                nc.sync.dma_start(out=t[:, 0:H], in_=kcT[c * P:(c + 1) * P,
                                                         0:H])
                kcT_sb.append(t)
            adjP_t = []
            for pp in range(NPAIR):
                t = mpool.tile([P, 2 * M], BF16, tag="adjP", bufs=4,
                               name=f"adjP{pp}")
                adjP_t.append(t)
                nc.sync.dma_start(out=t[:],
                                  in_=adjC[:, pp * 2 * M:(pp + 1) * 2 * M])
                if pp == 0:
                    for c in range(2):
                        nc.sync.dma_start(
                            out=kcT_sb[c][:, H:NKC],
                            in_=kcT[c * P:(c + 1) * P, H:NKC])
            for c in range(2):
                t = cpool.tile([P, D], BF16, tag=f"Em{c}")
                nc.sync.dma_start(out=t[:], in_=Em[c * P:(c + 1) * P, :])
                Em_sb.append(t)
            rdw_sb = cpool.tile([P, 4 * D], BF16, tag="rdw")
            nc.sync.dma_start(out=rdw_sb[:], in_=rdcat[:, :])
            rdbs_sb = cpool.tile([P, 4], F32, tag="rdbs")
            nc.sync.dma_start(out=rdbs_sb[:], in_=rdbs[:, :])

            warm = cpool.tile([P, 512], BF16, tag="warm")
            nc.vector.memset(warm[:], 0.0)
            ones1 = cpool.tile([1, P], BF16, tag="ones1")
            nc.vector.memset(ones1[:], 1.0)
            ones128 = cpool.tile([P, 1], BF16, tag="ones128")
            nc.vector.memset(ones128[:], 1.0)

            # ---- PE p-state warmup: ~3us of junk matmuls from t~0 so
            # the real setup/agg matmuls run at full clock
            with tc.tile_pool(name="warm_ps", bufs=1, space="PSUM") as wpool:
                wps = wpool.tile([1, 512], F32, tag="warm_ps")
                for i in range(7):
                    nc.tensor.matmul(wps[:], ones128[:], warm[:],
                                     start=(i == 0), stop=(i == 6))

            # ---- setup: exa1 row -> exa1b broadcast (bf16)
            exa1row = cpool.tile([1, M], BF16, tag="exa1row")
            exa1b = cpool.tile([P, M], BF16, tag="exa1b")
            exEhT = [cpool.tile([P, M], BF16, tag=f"exEhT{d}",
                                name=f"exEhT{d}") for d in (0, 1)]
            kcWh, kca2 = [], []
            with tc.tile_pool(name="setup_ps", bufs=2, space="PSUM") as spool:
                for b in range(3):
                    ms = slice(MOFF[b], MOFF[b] + MBS[b])
                    ps = spool.tile([1, MBS[b]], F32, tag="row_ps",
                                    name=f"row_ps{b}")
                    for c in range(2):
                        nc.tensor.matmul(ps[:], W1e_sb[c][:, D + 1:D + 2],
                                         exT_sb[c][:, ms],
                                         start=(c == 0), stop=(c == 1))
                    nc.vector.tensor_copy(exa1row[:, ms], ps[:])
                for b in range(3):
                    ms = slice(MOFF[b], MOFF[b] + MBS[b])
                    psb = spool.tile([P, MBS[b]], F32, tag="bc_ps",
                                     name=f"bc_ps{b}")
                    nc.tensor.matmul(psb[:], ones1[:], exa1row[:, ms],
                                     start=True, stop=True)
                    if b == 0:
                        nc.scalar.copy(exa1b[:, ms], psb[:])
                    else:
                        nc.vector.tensor_copy(exa1b[:, ms], psb[:])
                # kcWh chunks (bf16) + kca2 bias columns (f32); second
                # half is emitted mid-loop to unblock the first aggs
                for kk in range(KCH // 2):
                    ps = spool.tile([P, D + 2], F32, tag="kcwh_ps")
                    for c in range(2):
                        nc.tensor.matmul(
                            ps[:], kcT_sb[c][:, kk * P:(kk + 1) * P],
                            W1e_sb[c][:], start=(c == 0), stop=(c == 1))
                    t = cpool.tile([P, D], BF16, tag=f"kcWh{kk}",
                                   name=f"kcWh{kk}")
                    if kk % 2 == 0:
                        nc.vector.tensor_copy(t[:], ps[:, 0:D])
                    else:
                        nc.scalar.copy(t[:], ps[:, 0:D])
                    kcWh.append(t)
                    tb = cpool.tile([P, 1], F32, tag=f"kca2_{kk}",
                                    name=f"kca2_{kk}")
                    nc.vector.tensor_copy(tb[:], ps[:, D:D + 1])
                    kca2.append(tb)

            for kk in range(KCH // 2, KCH):
                t = cpool.tile([P, D], BF16, tag=f"kcWh{kk}",
                               name=f"kcWh{kk}b")
                kcWh.append(t)
                tb = cpool.tile([P, 1], F32, tag=f"kca2_{kk}",
                                name=f"kca2_{kk}b")
                kca2.append(tb)

            # ---- psum accumulators: blocks 0,1 inline; block 2 reuses the
            # block-0 tags in a tail pass over the resident ptm tiles.
            apool_cm = tc.tile_pool(name="acc_ps", bufs=1, space="PSUM")
            apool = apool_cm.__enter__()
            ehpool_cm = tc.tile_pool(name="eh_ps", bufs=1, space="PSUM")
            ehpool = ehpool_cm.__enter__()
            n0 = [apool.tile([P, MBS[b]], F32, tag=f"n0{b}",
                             name=f"n0_{b}") for b in range(3)]
            n1 = [apool.tile([P, MBS[b]], F32, tag=f"n1{b}",
                             name=f"n1_{b}") for b in range(3)]
            # psum budget: 6 n-tags + sSall + eh_ps = 8 banks; post psum
            # reuses the per-block n-tags after their readers finish
            sSall = apool.tile([P, 512], F32, tag="sSall")
            sS = [sSall[32 * b:32 * b + 1, 0:MBS[b]] for b in range(3)]

            # ---- main loop: per pair elementwise + blocks{0,1} inline
            ptmP = []
            for pp in range(NPAIR):
                adjP = adjP_t[pp]
                etP = mpool.tile([P, 2 * M], BF16, tag="etP")
                route = ROUTES[pp]
                for h in range(2):
                    kk = 2 * pp + h
                    hs = slice(h * M, (h + 1) * M)
                    if route == "A":
                        tmp = mpool.tile([P, M], BF16, tag="tmpA", bufs=4)
                        nc.vector.tensor_add(tmp[:], adjP[:, hs], exa1b[:])
                        nc.scalar.activation(etP[:, hs], tmp[:], AF.Prelu,
                                             bias=kca2[kk][:], alpha=ALPHA)
                    else:
                        tmp = mpool.tile([P, M], BF16, tag="tmpD", bufs=4)
                        nc.vector.scalar_tensor_tensor(
                            tmp[:], exa1b[:], kca2[kk][:], adjP[:, hs],
                            AluOpType.add, AluOpType.add)
                        nc.vector.scalar_tensor_tensor(
                            etP[:, hs], tmp[:], ALPHA, tmp[:],
                            AluOpType.mult, AluOpType.max)
                ptm = mpool.tile([P, 2 * M], BF16, tag="ptm", bufs=3,
                                 name=f"ptm{pp}")
                if pp >= NPAIR - 2:
                    nc.scalar.activation(ptm[:, 0:M], etP[:, 0:M], AF.Exp)
                    nc.scalar.activation(ptm[:, M:2 * M], etP[:, M:2 * M],
                                         AF.Exp)
                else:
                    nc.scalar.activation(ptm[:], etP[:], AF.Exp)
                if pp == 1:
                    for kk in range(KCH // 2, KCH):
                        psk = ehpool.tile([P, D + 2], F32, tag="eh_ps",
                                          name=f"kcwh_ps{kk}")
                        for c in range(2):
                            nc.tensor.matmul(
                                psk[:], kcT_sb[c][:, kk * P:(kk + 1) * P],
                                W1e_sb[c][:], start=(c == 0), stop=(c == 1))
                        if kk % 2 == 0:
                            nc.vector.tensor_copy(kcWh[kk][:], psk[:, 0:D])
                        else:
                            nc.scalar.copy(kcWh[kk][:], psk[:, 0:D])
                        nc.vector.tensor_copy(kca2[kk][:], psk[:, D:D + 1])
                if pp == 2:
                    for d in range(2):
                        for b in range(3):
                            ms = slice(MOFF[b], MOFF[b] + MBS[b])
                            pse = ehpool.tile([P, MBS[b]], F32, tag="eh_ps",
                                              name=f"eh_ps{b}_{d}")
                            for c in range(2):
                                nc.tensor.matmul(
                                    pse[:], Em_sb[c][:, d * P:(d + 1) * P],
                                    exT_sb[c][:, ms], start=(c == 0),
                                    stop=(c == 1))
                            if (d + b) % 2 == 0:
                                nc.scalar.copy(exEhT[d][:, ms], pse[:])
                            else:
                                nc.vector.tensor_copy(exEhT[d][:, ms],
                                                      pse[:])
                ptmP.append(ptm)
                st, sp = (pp == 0), (pp == NPAIR - 1)
                for h in range(2):
                    kk = 2 * pp + h
                    for b in range(2):
                        ms = slice(h * M + MOFF[b], h * M + MOFF[b] + MBS[b])
                        nc.tensor.matmul(n0[b][:], kcWh[kk][:, 0:P],
                                         ptm[:, ms], start=(st and h == 0),
                                         stop=(sp and h == 1))
                        nc.tensor.matmul(n1[b][:], kcWh[kk][:, P:2 * P],
                                         ptm[:, ms], start=(st and h == 0),
                                         stop=(sp and h == 1))
                        nc.tensor.matmul(sS[b], ones128[:], ptm[:, ms],
                                         start=(st and h == 0),
                                         stop=(sp and h == 1))

            # block-2 aggregation as a second pass over resident ptm: runs
            # on PE while posts(0,1) occupy the other engines
            def blk2_aggs(prange):
                for pp in prange:
                    st, sp = (pp == 0), (pp == NPAIR - 1)
                    for h in range(2):
                        kk = 2 * pp + h
                        ms = slice(h * M + MOFF[2],
                                   h * M + MOFF[2] + MBS[2])
                        nc.tensor.matmul(n0[2][:], kcWh[kk][:, 0:P],
                                         ptmP[pp][:, ms],
                                         start=(st and h == 0),
                                         stop=(sp and h == 1))
                        nc.tensor.matmul(n1[2][:], kcWh[kk][:, P:2 * P],
                                         ptmP[pp][:, ms],
                                         start=(st and h == 0),
                                         stop=(sp and h == 1))
                        nc.tensor.matmul(sS[2], ones128[:], ptmP[pp][:, ms],
                                         start=(st and h == 0),
                                         stop=(sp and h == 1))
            blk2_aggs(range(0, 4))

            # ---- post stage for a finished block
            res = [cpool.tile([P, M], BF16, tag=f"res{oo}",
                              name=f"res{oo}") for oo in (0, 1)]

            def post(b, n0t, n1t, sSt):
                mb = MBS[b]
                ms = slice(MOFF[b], MOFF[b] + mb)
                srow = qpool.tile([1, mb], BF16, tag="srow", bufs=3)
                with nc.allow_low_precision(reason="f32r storage is f32"):
                    nc.vector.reciprocal(srow[:], sSt)
                sbps = apool.tile([P, mb], F32, tag=f"n0{b}",
                                  name=f"sb_ps{b}")
                nc.tensor.matmul(sbps[:], ones1[:], srow[:],
                                 start=True, stop=True)
                sinvb = qpool.tile([P, mb], F32, tag="sinvb", bufs=3)
                nc.scalar.copy(sinvb[:], sbps[:])
                nk0 = qpool.tile([P, mb], BF16, tag="nk0", bufs=3)
                nc.scalar.copy(nk0[:], n0t[:])
                nk1 = qpool.tile([P, mb], BF16, tag="nk1", bufs=3)
                if b == 0:
                    nc.vector.tensor_copy(nk1[:], n1t[:])
                else:
                    nc.scalar.copy(nk1[:], n1t[:])
                t0 = qpool.tile([P, mb], BF16, tag="t0", bufs=3)
                nc.gpsimd.tensor_mul(t0[:], nk0[:], exEhT[0][:, ms])
                t1 = qpool.tile([P, mb], BF16, tag="t1", bufs=3)
                nc.gpsimd.tensor_mul(t1[:], nk1[:], exEhT[1][:, ms])
                feat = [nk0, nk1, t0, t1]
                for oo in range(2):
                    ups = apool.tile([P, mb], F32,
                                      tag=f"n1{b}" if oo == 0 else f"n0{b}",
                                      name=f"u_ps{b}_{oo}")
                    for dd in range(4):
                        nc.tensor.matmul(
                            ups[:],
                            rdw_sb[:, dd * D + oo * P:dd * D + oo * P + P],
                            feat[dd][:], start=(dd == 0), stop=(dd == 3))
                    prod = qpool.tile([P, mb], BF16, tag="prod", bufs=3)
                    nc.vector.tensor_mul(prod[:], ups[:], sinvb[:])
                    # r1m1 = max(prod + (b-1), -1) = relu(y) - 1  (DVE)
                    r1m1 = qpool.tile([P, mb], BF16, tag="r1m1", bufs=3)
                    nc.vector.tensor_scalar(r1m1[:], prod[:],
                                            rdbs_sb[:, 2 + oo:3 + oo], -1.0,
                                            AluOpType.add, AluOpType.max)
                    # res = elu(y) = min(exp(y),1) + r1m1
                    if oo == 0 or b == 2:
                        r2 = qpool.tile([P, mb], BF16, tag="r2", bufs=3)
                        nc.scalar.activation(r2[:], prod[:], AF.Exp,
                                             bias=rdbs_sb[:, oo:oo + 1])
                        nc.vector.scalar_tensor_tensor(
                            res[oo][:, ms], r2[:], 1.0, r1m1[:],
                            AluOpType.min, AluOpType.add)
                    else:
                        # min(exp(y),1) = exp(min(y,0)); Pool does the add
                        tmin = qpool.tile([P, mb], BF16, tag="tmin", bufs=3)
                        nc.vector.tensor_scalar(tmin[:], prod[:],
                                                rdbs_sb[:, oo:oo + 1], 0.0,
                                                AluOpType.add, AluOpType.min)
                        r2c = qpool.tile([P, mb], BF16, tag="r2c", bufs=3)
                        nc.scalar.activation(r2c[:], tmin[:], AF.Exp)
                        nc.gpsimd.tensor_add(res[oo][:, ms], r2c[:],
                                             r1m1[:])
                    nc.sync.dma_start(out=outT[oo * P:(oo + 1) * P, ms],
                                      in_=res[oo][:, ms])

            post(0, n0[0], n1[0], sS[0])
            blk2_aggs(range(4, NPAIR))
            post(1, n0[1], n1[1], sS[1])
            post(2, n0[2], n1[2], sS[2])
            ehpool_cm.__exit__(None, None, None)
            apool_cm.__exit__(None, None, None)
    nc.finalize()
    return nc


_PROGRAM = None


def _get_program():
    global _PROGRAM
    if _PROGRAM is None:
        _PROGRAM = _build()
    return _PROGRAM


def _in_maps(exercise_h, kc_h, adj, W1, E, a, rd_w, rd_b):
    f = np.float32
    bf = ml_dtypes.bfloat16
    a1 = np.ascontiguousarray(a[:D, 0], dtype=f)
    a2 = np.ascontiguousarray(a[D:, 0], dtype=f)
    W1 = np.asarray(W1, dtype=f)
    W1e = np.concatenate([W1, (W1 @ a2)[:, None], (W1 @ a1)[:, None]],
                         axis=1)                               # [256, 258]
    kcT = np.zeros((2 * P, NKC), dtype=bf)
    kcT[:, :2000] = np.asarray(kc_h, dtype=f).T
    Em = np.ascontiguousarray(np.asarray(E, dtype=bf))
    rdwT = np.asarray(rd_w, dtype=f).T                         # [512, 256]
    rdcat = np.zeros((P, 4 * D), dtype=bf)
    for dd in range(4):
        rdcat[:, dd * D:(dd + 1) * D] = rdwT[dd * P:(dd + 1) * P, :]
    rdb = np.asarray(rd_b, dtype=f)
    rdbs = np.zeros((P, 4), dtype=f)
    rdbs[:, 0] = rdb[0:P]
    rdbs[:, 1] = rdb[P:2 * P]
    rdbs[:, 2] = rdb[0:P] - 1.0
    rdbs[:, 3] = rdb[P:2 * P] - 1.0
    shared = {"kcT": kcT, "W1e": np.ascontiguousarray(W1e.astype(bf)), "Em": Em,
              "rdcat": np.ascontiguousarray(rdcat),
              "rdbs": np.ascontiguousarray(rdbs)}
    maps = []
    for c in range(NCORES):
        sl = slice(c * ROWS, (c + 1) * ROWS)
        exT_c = np.ascontiguousarray(
            np.asarray(exercise_h[sl], dtype=f).T.astype(bf))
        adjx = np.asarray(adj[sl], dtype=f).T                  # [2000, 1250]
        adjC_c = np.full((P, KCH * M), -100.0, dtype=bf)
        for kk in range(KCH):
            nreal = max(0, min(2000 - kk * P, P))
            blk = np.full((P, M), -100.0, dtype=f)
            blk[:nreal, :] = (adjx[kk * P:kk * P + nreal] - 1.0) * 100.0
            adjC_c[:, kk * M:(kk + 1) * M] = blk
        del adjx
        maps.append({"exT": exT_c, "adjC": adjC_c, **shared})
    return maps


def kernel(exercise_h, kc_h, adj, W1, E, a, rd_w, rd_b):
    nc = _get_program()
    maps = _in_maps(exercise_h, kc_h, adj, W1, E, a, rd_w, rd_b)
    res = run_bass_kernel_spmd(nc, maps, list(range(NCORES))).results
    out = np.empty((N_E, D), dtype=np.float32)
    for c in range(NCORES):
        r = res[c]["outT"].astype(np.float32)
        out[c * ROWS:(c + 1) * ROWS] = r.T
    return out


# revision 47
# speedup vs baseline: 1.3439x; 1.0198x over previous
"""GAT-style graph encoder on 8 trn2 NeuronCores.

Reference computation (per exercise row i over kc nodes j):
    kc_Wh = kc_h @ W1; ex_Wh = ex_h @ W1
    e[i,j] = leaky_relu(ex_Wh[i]@a1 + kc_Wh[j]@a2, 0.2)
    att = softmax(where(adj>0, e, -9e15), axis=1)
    new_kc = att @ kc_Wh; ex_Eh = ex_h @ E
    out = elu(concat([new_kc, new_kc*ex_Eh]) @ rd_w.T + rd_b)

Strategy: row-shard exercises over 8 cores (1250 cols each, no padding).
On-chip layout is transposed ([kc_or_feature, exercise]) so softmax
numerator/denominator are PE matmuls contracting over the kc partition
axis.  Masking is additive (adj shipped as a 0/-100 bf16 logit fold);
softmax runs without max-subtraction (logits bounded, exp in f32->bf16).
All operands are bf16 (rel-err budget ~0.7% << 2e-2 tolerance).

Per kc chunk-pair (2x128 kc rows x 1250 ex cols):
  - one 2500-wide DMA for the fold mask (8 adj loads total; HWDGE is a
    serial 625ns/DMA device, so few fat DMAs beat many thin ones)
  - route A: fold-add on DVE tt (2x bf16 mode) or Pool tt (one half
    each), leaky via ACT Prelu with the kca2 bias port
  - route D: DVE stt carries exa1b+kca2+fold, second DVE stt the leaky
    ((x*0.2) max x); ACT only does exp.  A/D mix balances ACT vs DVE.
  - one 2500-wide ACT Exp into bf16 ptm tiles (last pairs split in two
    to shorten the tail)
  - 3 bf16 matmul streams (n0, n1, denominator) accumulate psum blocks
    {0,1} inline; block {2} is a second pass over resident ptm tiles so
    its PE work overlaps the posts of blocks {0,1}.  The 3 denominator
    rows share one psum bank at partition offsets 0/32/64.
PE p-state is warmed with junk matmuls at t=0 (full 2.4GHz clock needs
~3us of continuous busy).  Post stage normalizes late (1/s applied to
the readout psum), elu(y) = min(exp(y),1) + (relu(y)-1) with engine-
legal op placement (Pool cannot touch PSUM or run TensorScalarPtr).
Setup copies and post ops are spread ACT/DVE/Pool to balance occupancy.
HW exec (TimelineSim): 60.8us vs 81.8us baseline; rel err 6.6e-3.
"""

import ml_dtypes
import numpy as np

import concourse.bacc as bacc
import concourse.mybir as mybir
from concourse.alu_op_type import AluOpType
from concourse.bass_utils import run_bass_kernel_spmd
from concourse.tile import TileContext

F32 = mybir.dt.float32
F32R = mybir.dt.float32r
BF16 = mybir.dt.bfloat16
AF = mybir.ActivationFunctionType

P = 128
D = 256
NKC = 2048
KCH = 16                    # kc chunks of 128
NPAIR = 8                   # chunk pairs
M = 1250                    # exercise cols per core (no padding)
MBS = (512, 512, 226)
MOFF = (0, 512, 1024)
NCORES = 8
ROWS = 1250
N_E = 10000
ALPHA = 0.2
# Per-pair elementwise route: A = ACT Prelu carries kca2+leaky (DVE does
# the fold add); D = DVE stt carries fold+kca2 and a second stt the leaky
# (ACT only does exp).  Mix balances ACT vs DVE occupancy.
ROUTES = ("A", "A", "A", "A", "A", "D", "A", "D")


def _build():
    nc = bacc.Bacc("TRN2", target_bir_lowering=False, debug=False,
                   num_devices=NCORES)
    exT = nc.declare_dram_parameter("exT", [2 * P, M], BF16, isOutput=False)
    adjC = nc.declare_dram_parameter("adjC", [P, KCH * M], BF16,
                                     isOutput=False)
    kcT = nc.declare_dram_parameter("kcT", [2 * P, NKC], BF16, isOutput=False)
    W1e = nc.declare_dram_parameter("W1e", [2 * P, D + 2], BF16,
                                    isOutput=False)
    Em = nc.declare_dram_parameter("Em", [2 * P, D], BF16, isOutput=False)
    rdcat = nc.declare_dram_parameter("rdcat", [P, 4 * D], BF16,
                                      isOutput=False)
    rdbs = nc.declare_dram_parameter("rdbs", [P, 4], F32, isOutput=False)
    outT = nc.declare_dram_parameter("outT", [2 * P, M], BF16, isOutput=True)

    with TileContext(nc) as tc:
        with tc.tile_pool(name="const", bufs=1) as cpool, \
             tc.tile_pool(name="mwork", bufs=3) as mpool, \
             tc.tile_pool(name="post", bufs=2) as qpool:
            # ---- const loads (ordered so compute can start early:
            # exT/W1e gate exa1b; first adj pairs gate the elementwise)
            kcT_sb, W1e_sb, Em_sb, exT_sb = [], [], [], []
            for c in range(2):
                t = cpool.tile([P, M], BF16, tag=f"exT{c}")
                nc.sync.dma_start(out=t[:], in_=exT[c * P:(c + 1) * P, :])
                exT_sb.append(t)
                t = cpool.tile([P, D + 2], BF16, tag=f"W1e{c}")
                nc.sync.dma_start(out=t[:], in_=W1e[c * P:(c + 1) * P, :])
                W1e_sb.append(t)
            H = NKC // 2
            for c in range(2):
                t = cpool.tile([P, NKC], BF16, tag=f"kcT{c}")
                nc.sync.dma_start(out=t[:, 0:H], in_=kcT[c * P:(c + 1) * P,
                                                         0:H])
                kcT_sb.append(t)
            adjP_t = []
            for pp in range(NPAIR):
                t = mpool.tile([P, 2 * M], BF16, tag="adjP", bufs=4,
                               name=f"adjP{pp}")
                adjP_t.append(t)
                nc.sync.dma_start(out=t[:],
                                  in_=adjC[:, pp * 2 * M:(pp + 1) * 2 * M])
                if pp == 0:
                    for c in range(2):
                        nc.sync.dma_start(
                            out=kcT_sb[c][:, H:NKC],
                            in_=kcT[c * P:(c + 1) * P, H:NKC])
            for c in range(2):
                t = cpool.tile([P, D], BF16, tag=f"Em{c}")
                nc.sync.dma_start(out=t[:], in_=Em[c * P:(c + 1) * P, :])
                Em_sb.append(t)
            rdw_sb = cpool.tile([P, 4 * D], BF16, tag="rdw")
            nc.sync.dma_start(out=rdw_sb[:], in_=rdcat[:, :])
            rdbs_sb = cpool.tile([P, 4], F32, tag="rdbs")
            nc.sync.dma_start(out=rdbs_sb[:], in_=rdbs[:, :])

            warm = cpool.tile([P, 512], BF16, tag="warm")
            nc.vector.memset(warm[:], 0.0)
            ones1 = cpool.tile([1, P], BF16, tag="ones1")
            nc.vector.memset(ones1[:], 1.0)
            ones128 = cpool.tile([P, 1], BF16, tag="ones128")
            nc.vector.memset(ones128[:], 1.0)

            # ---- PE p-state warmup: ~3us of junk matmuls from t~0 so
            # the real setup/agg matmuls run at full clock
            with tc.tile_pool(name="warm_ps", bufs=1, space="PSUM") as wpool:
                wps = wpool.tile([1, 512], F32, tag="warm_ps")
                for i in range(7):
                    nc.tensor.matmul(wps[:], ones128[:], warm[:],
                                     start=(i == 0), stop=(i == 6))

            # ---- setup: exa1 row -> exa1b broadcast (bf16)
            exa1row = cpool.tile([1, M], BF16, tag="exa1row")
            exa1b = cpool.tile([P, M], BF16, tag="exa1b")
            exEhT = [cpool.tile([P, M], BF16, tag=f"exEhT{d}",
                                name=f"exEhT{d}") for d in (0, 1)]
            kcWh, kca2 = [], []
            with tc.tile_pool(name="setup_ps", bufs=2, space="PSUM") as spool:
                for b in range(3):
                    ms = slice(MOFF[b], MOFF[b] + MBS[b])
                    ps = spool.tile([1, MBS[b]], F32, tag="row_ps",
                                    name=f"row_ps{b}")
                    for c in range(2):
                        nc.tensor.matmul(ps[:], W1e_sb[c][:, D + 1:D + 2],
                                         exT_sb[c][:, ms],
                                         start=(c == 0), stop=(c == 1))
                    nc.vector.tensor_copy(exa1row[:, ms], ps[:])
                for b in range(3):
                    ms = slice(MOFF[b], MOFF[b] + MBS[b])
                    psb = spool.tile([P, MBS[b]], F32, tag="bc_ps",
                                     name=f"bc_ps{b}")
                    nc.tensor.matmul(psb[:], ones1[:], exa1row[:, ms],
                                     start=True, stop=True)
                    if b == 0:
                        nc.scalar.copy(exa1b[:, ms], psb[:])
                    else:
                        nc.vector.tensor_copy(exa1b[:, ms], psb[:])
                # kcWh chunks (bf16) + kca2 bias columns (f32); second
                # half is emitted mid-loop to unblock the first aggs
                for kk in range(KCH // 2):
                    ps = spool.tile([P, D + 2], F32, tag="kcwh_ps")
                    for c in range(2):
                        nc.tensor.matmul(
                            ps[:], kcT_sb[c][:, kk * P:(kk + 1) * P],
                            W1e_sb[c][:], start=(c == 0), stop=(c == 1))
                    t = cpool.tile([P, D], BF16, tag=f"kcWh{kk}",
                                   name=f"kcWh{kk}")
                    if kk % 2 == 0:
                        nc.vector.tensor_copy(t[:], ps[:, 0:D])
                    else:
                        nc.scalar.copy(t[:], ps[:, 0:D])
                    kcWh.append(t)
                    tb = cpool.tile([P, 1], F32, tag=f"kca2_{kk}",
                                    name=f"kca2_{kk}")
                    nc.vector.tensor_copy(tb[:], ps[:, D:D + 1])
                    kca2.append(tb)

            for kk in range(KCH // 2, KCH):
                t = cpool.tile([P, D], BF16, tag=f"kcWh{kk}",
                               name=f"kcWh{kk}b")
                kcWh.append(t)
                tb = cpool.tile([P, 1], F32, tag=f"kca2_{kk}",
                                name=f"kca2_{kk}b")
                kca2.append(tb)

            # ---- psum accumulators: blocks 0,1 inline; block 2 reuses the
            # block-0 tags in a tail pass over the resident ptm tiles.
            apool_cm = tc.tile_pool(name="acc_ps", bufs=1, space="PSUM")
            apool = apool_cm.__enter__()
            ehpool_cm = tc.tile_pool(name="eh_ps", bufs=1, space="PSUM")
            ehpool = ehpool_cm.__enter__()
            n0 = [apool.tile([P, MBS[b]], F32, tag=f"n0{b}",
                             name=f"n0_{b}") for b in range(3)]
            n1 = [apool.tile([P, MBS[b]], F32, tag=f"n1{b}",
                             name=f"n1_{b}") for b in range(3)]
            # psum budget: 6 n-tags + sSall + eh_ps = 8 banks; post psum
            # reuses the per-block n-tags after their readers finish
            sSall = apool.tile([P, 512], F32, tag="sSall")
            sS = [sSall[32 * b:32 * b + 1, 0:MBS[b]] for b in range(3)]

            # ---- main loop: per pair elementwise + blocks{0,1} inline
            ptmP = []
            for pp in range(NPAIR):
                adjP = adjP_t[pp]
                etP = mpool.tile([P, 2 * M], BF16, tag="etP")
                route = ROUTES[pp]
                for h in range(2):
                    kk = 2 * pp + h
                    hs = slice(h * M, (h + 1) * M)
                    if route == "A":
                        tmp = mpool.tile([P, M], BF16, tag="tmpA", bufs=4)
                        nc.vector.tensor_add(tmp[:], adjP[:, hs], exa1b[:])
                        nc.scalar.activation(etP[:, hs], tmp[:], AF.Prelu,
                                             bias=kca2[kk][:], alpha=ALPHA)
                    else:
                        tmp = mpool.tile([P, M], BF16, tag="tmpD", bufs=4)
                        nc.vector.scalar_tensor_tensor(
                            tmp[:], exa1b[:], kca2[kk][:], adjP[:, hs],
                            AluOpType.add, AluOpType.add)
                        nc.vector.scalar_tensor_tensor(
                            etP[:, hs], tmp[:], ALPHA, tmp[:],
                            AluOpType.mult, AluOpType.max)
                ptm = mpool.tile([P, 2 * M], BF16, tag="ptm", bufs=3,
                                 name=f"ptm{pp}")
                if pp >= NPAIR - 2:
                    nc.scalar.activation(ptm[:, 0:M], etP[:, 0:M], AF.Exp)
                    nc.scalar.activation(ptm[:, M:2 * M], etP[:, M:2 * M],
                                         AF.Exp)
                else:
                    nc.scalar.activation(ptm[:], etP[:], AF.Exp)
                if pp == 1:
                    for kk in range(KCH // 2, KCH):
                        psk = ehpool.tile([P, D + 2], F32, tag="eh_ps",
                                          name=f"kcwh_ps{kk}")
                        for c in range(2):
                            nc.tensor.matmul(
                                psk[:], kcT_sb[c][:, kk * P:(kk + 1) * P],
                                W1e_sb[c][:], start=(c == 0), stop=(c == 1))
                        if kk % 2 == 0:
                            nc.vector.tensor_copy(kcWh[kk][:], psk[:, 0:D])
                        else:
                            nc.scalar.copy(kcWh[kk][:], psk[:, 0:D])
                        nc.vector.tensor_copy(kca2[kk][:], psk[:, D:D + 1])
                if pp == 2:
                    for d in range(2):
                        for b in range(3):
                            ms = slice(MOFF[b], MOFF[b] + MBS[b])
                            pse = ehpool.tile([P, MBS[b]], F32, tag="eh_ps",
                                              name=f"eh_ps{b}_{d}")
                            for c in range(2):
                                nc.tensor.matmul(
                                    pse[:], Em_sb[c][:, d * P:(d + 1) * P],
                                    exT_sb[c][:, ms], start=(c == 0),
                                    stop=(c == 1))
                            if (d + b) % 2 == 0:
                                nc.scalar.copy(exEhT[d][:, ms], pse[:])
                            else:
                                nc.vector.tensor_copy(exEhT[d][:, ms],
                                                      pse[:])
                ptmP.append(ptm)
                st, sp = (pp == 0), (pp == NPAIR - 1)
                for h in range(2):
                    kk = 2 * pp + h
                    for b in range(2):
                        ms = slice(h * M + MOFF[b], h * M + MOFF[b] + MBS[b])
                        nc.tensor.matmul(n0[b][:], kcWh[kk][:, 0:P],
                                         ptm[:, ms], start=(st and h == 0),
                                         stop=(sp and h == 1))
                        nc.tensor.matmul(n1[b][:], kcWh[kk][:, P:2 * P],
                                         ptm[:, ms], start=(st and h == 0),
                                         stop=(sp and h == 1))
                        nc.tensor.matmul(sS[b], ones128[:], ptm[:, ms],
                                         start=(st and h == 0),
                                         stop=(sp and h == 1))

            # block-2 aggregation as a second pass over resident ptm: runs
            # on PE while posts(0,1) occupy the other engines
            def blk2_aggs(prange):
                for pp in prange:
                    st, sp = (pp == 0), (pp == NPAIR - 1)
                    for h in range(2):
                        kk = 2 * pp + h
                        ms = slice(h * M + MOFF[2],
                                   h * M + MOFF[2] + MBS[2])
                        nc.tensor.matmul(n0[2][:], kcWh[kk][:, 0:P],
                                         ptmP[pp][:, ms],
                                         start=(st and h == 0),
                                         stop=(sp and h == 1))
                        nc.tensor.matmul(n1[2][:], kcWh[kk][:, P:2 * P],
                                         ptmP[pp][:, ms],
                                         start=(st and h == 0),
                                         stop=(sp and h == 1))
                        nc.tensor.matmul(sS[2], ones128[:], ptmP[pp][:, ms],
                                         start=(st and h == 0),
                                         stop=(sp and h == 1))
            blk2_aggs(range(0, 4))

            # ---- post stage for a finished block
            res = [cpool.tile([P, M], BF16, tag=f"res{oo}",
                              name=f"res{oo}") for oo in (0, 1)]

            def post(b, n0t, n1t, sSt):
                mb = MBS[b]
                ms = slice(MOFF[b], MOFF[b] + mb)
                srow = qpool.tile([1, mb], BF16, tag="srow", bufs=3)
                with nc.allow_low_precision(reason="f32r storage is f32"):
                    nc.vector.reciprocal(srow[:], sSt)
                sbps = apool.tile([P, mb], F32, tag=f"n0{b}",
                                  name=f"sb_ps{b}")
                nc.tensor.matmul(sbps[:], ones1[:], srow[:],
                                 start=True, stop=True)
                sinvb = qpool.tile([P, mb], F32, tag="sinvb", bufs=3)
                nc.scalar.copy(sinvb[:], sbps[:])
                nk0 = qpool.tile([P, mb], BF16, tag="nk0", bufs=3)
                nc.scalar.copy(nk0[:], n0t[:])
                nk1 = qpool.tile([P, mb], BF16, tag="nk1", bufs=3)
                if b == 0:
                    nc.vector.tensor_copy(nk1[:], n1t[:])
                else:
                    nc.scalar.copy(nk1[:], n1t[:])
                t0 = qpool.tile([P, mb], BF16, tag="t0", bufs=3)
                nc.gpsimd.tensor_mul(t0[:], nk0[:], exEhT[0][:, ms])
                t1 = qpool.tile([P, mb], BF16, tag="t1", bufs=3)
                nc.gpsimd.tensor_mul(t1[:], nk1[:], exEhT[1][:, ms])
                feat = [nk0, nk1, t0, t1]
                for oo in range(2):
                    ups = apool.tile([P, mb], F32,
                                      tag=f"n1{b}" if oo == 0 else f"n0{b}",
                                      name=f"u_ps{b}_{oo}")
                    for dd in range(4):
                        nc.tensor.matmul(
                            ups[:],
                            rdw_sb[:, dd * D + oo * P:dd * D + oo * P + P],
                            feat[dd][:], start=(dd == 0), stop=(dd == 3))
                    prod = qpool.tile([P, mb], BF16, tag="prod", bufs=3)
                    nc.vector.tensor_mul(prod[:], ups[:], sinvb[:])
                    # r1m1 = max(prod + (b-1), -1) = relu(y) - 1  (DVE)
                    r1m1 = qpool.tile([P, mb], BF16, tag="r1m1", bufs=3)
                    nc.vector.tensor_scalar(r1m1[:], prod[:],
                                            rdbs_sb[:, 2 + oo:3 + oo], -1.0,
                                            AluOpType.add, AluOpType.max)
                    # res = elu(y) = min(exp(y),1) + r1m1
                    r2 = qpool.tile([P, mb], BF16, tag="r2", bufs=3)
                    nc.scalar.activation(r2[:], prod[:], AF.Exp,
                                         bias=rdbs_sb[:, oo:oo + 1])
                    nc.vector.scalar_tensor_tensor(
                        res[oo][:, ms], r2[:], 1.0, r1m1[:],
                        AluOpType.min, AluOpType.add)
                    nc.sync.dma_start(out=outT[oo * P:(oo + 1) * P, ms],
                                      in_=res[oo][:, ms])

            post(0, n0[0], n1[0], sS[0])
            blk2_aggs(range(4, NPAIR))
            post(1, n0[1], n1[1], sS[1])
            post(2, n0[2], n1[2], sS[2])
            ehpool_cm.__exit__(None, None, None)
            apool_cm.__exit__(None, None, None)
    nc.finalize()
    return nc


_PROGRAM = None


def _get_program():
    global _PROGRAM
    if _PROGRAM is None:
        _PROGRAM = _build()
    return _PROGRAM


def _in_maps(exercise_h, kc_h, adj, W1, E, a, rd_w, rd_b):
    f = np.float32
    bf = ml_dtypes.bfloat16
    a1 = np.ascontiguousarray(a[:D, 0], dtype=f)
    a2 = np.ascontiguousarray(a[D:, 0], dtype=f)
    W1 = np.asarray(W1, dtype=f)
    W1e = np.concatenate([W1, (W1 @ a2)[:, None], (W1 @ a1)[:, None]],
                         axis=1)                               # [256, 258]
    kcT = np.zeros((2 * P, NKC), dtype=bf)
    kcT[:, :2000] = np.asarray(kc_h, dtype=f).T
    Em = np.ascontiguousarray(np.asarray(E, dtype=bf))
    rdwT = np.asarray(rd_w, dtype=f).T                         # [512, 256]
    rdcat = np.zeros((P, 4 * D), dtype=bf)
    for dd in range(4):
        rdcat[:, dd * D:(dd + 1) * D] = rdwT[dd * P:(dd + 1) * P, :]
    rdb = np.asarray(rd_b, dtype=f)
    rdbs = np.zeros((P, 4), dtype=f)
    rdbs[:, 0] = rdb[0:P]
    rdbs[:, 1] = rdb[P:2 * P]
    rdbs[:, 2] = rdb[0:P] - 1.0
    rdbs[:, 3] = rdb[P:2 * P] - 1.0
    shared = {"kcT": kcT, "W1e": np.ascontiguousarray(W1e.astype(bf)), "Em": Em,
              "rdcat": np.ascontiguousarray(rdcat),
              "rdbs": np.ascontiguousarray(rdbs)}
    maps = []
    for c in range(NCORES):
        sl = slice(c * ROWS, (c + 1) * ROWS)
        exT_c = np.ascontiguousarray(
            np.asarray(exercise_h[sl], dtype=f).T.astype(bf))
        adjx = np.asarray(adj[sl], dtype=f).T                  # [2000, 1250]
        adjC_c = np.full((P, KCH * M), -100.0, dtype=bf)
        for kk in range(KCH):
            nreal = max(0, min(2000 - kk * P, P))
            blk = np.full((P, M), -100.0, dtype=f)
            blk[:nreal, :] = (adjx[kk * P:kk * P + nreal] - 1.0) * 100.0
            adjC_c[:, kk * M:(kk + 1) * M] = blk
        del adjx
        maps.append({"exT": exT_c, "adjC": adjC_c, **shared})
    return maps


def kernel(exercise_h, kc_h, adj, W1, E, a, rd_w, rd_b):
    nc = _get_program()
    maps = _in_maps(exercise_h, kc_h, adj, W1, E, a, rd_w, rd_b)
    res = run_bass_kernel_spmd(nc, maps, list(range(NCORES))).results
    out = np.empty((N_E, D), dtype=np.float32)
    for c in range(NCORES):
        r = res[c]["outT"].astype(np.float32)
        out[c * ROWS:(c + 1) * ROWS] = r.T
    return out
